# revision 8
# baseline (speedup 1.0000x reference)
"""Trainium2 Bass kernel for conv3x3(2048->256)+BN+ReLU -> 1x1 projections ->
row-slice attention (rows 10:20) -> scatter+residual.

Strategy: 8 NeuronCores, data-parallel over (batch b in 0..3) x (H-half in 0..1).
Each core computes a [256, 32, 128] slab of r = relu(BN(conv3x3(x))) via
PSUM-accumulated matmuls (16 C_in chunks x 9 taps, split into two 8-chunk
passes to fit weights in SBUF), the q/k projections for its rows, and the
v projection + full attention for its LOCAL rows 10:20 (only valid/used on
H-half 0; running it everywhere keeps one uniform SPMD program and balances
load). All matmuls run in float32r (fp32 with 11-bit RNE mantissa, full PE
rate); inputs are pre-rounded on the host so no on-device rounding pass is
needed. BN scale is folded into the conv weights on the host; BN shift is
applied as the ReLU activation bias.
"""

import json
import sys

sys.path.insert(0, "/opt/trn_rl_repo")

import numpy as np

# ---------------- problem constants (hardcoded per spec) ----------------
B = 4
CI = 2048
H = 64
W = 128
CO = 256
EPS = 1e-5
P = 128
KC = CI // P  # 16 ci chunks
KCH = KC // 2  # chunks per pass
NTAP = 9
HH = H // 2  # 32 output rows per core
RIN = HH + 2  # padded input rows per core
WP = W + 2  # padded width
NA = 10 * W  # 1280 attention tokens
AQ = 64  # q/k channels
NCORES = 8

# ---------------- fp32r host-side rounding (11-bit RNE mantissa) --------


def _round_f32r(a):
    b = np.ascontiguousarray(a, dtype=np.float32).view(np.uint32).astype(np.uint64)
    drop = 12
    lsb = (b >> drop) & 1
    r = ((b + (1 << (drop - 1)) - 1 + lsb) >> drop) << drop
    return r.astype(np.uint32).view(np.float32).reshape(a.shape)


# ---------------- walrus single-wait workaround -------------------------
# The walrus build in this container rejects instructions carrying more than
# one semaphore wait. Split k-wait instructions into (k-1) single-wait NoOps
# on the same engine followed by the original instruction with one wait.


def _split_multiwait(m):
    ctr = 0
    for fn in m.get("functions", []):
        for blk in fn.get("blocks", []):
            out = []
            for inst in blk.get("instructions", []):
                si = inst.get("sync_info")
                waits = si.get("on_wait") if isinstance(si, dict) else None
                if waits and len(waits) > 1:
                    for w in waits[:-1]:
                        out.append(
                            {
                                "debug": inst.get("debug", 0),
                                "engine": inst["engine"],
                                "ins": [],
                                "name": f"splitw-{ctr}",
                                "opcode": "NoOp",
                                "outs": [],
                                "text_hint": "splitwait",
                                "sync_info": {"on_update": [], "on_wait": [w]},
                            }
                        )
                        ctr += 1
                    si["on_wait"] = [waits[-1]]
                out.append(inst)
            blk["instructions"] = out
    return m


_patched = False


def _apply_patch():
    global _patched
    if _patched:
        return
    import concourse.bass as bass_mod

    orig = bass_mod.Bass.to_json_bytes

    def patched(self, *args, **kwargs):
        raw = orig(self, *args, **kwargs)
        return json.dumps(_split_multiwait(json.loads(raw))).encode()

    bass_mod.Bass.to_json_bytes = patched
    _patched = True


# ---------------- device program ----------------------------------------

_prog_cache = {}


def _build_program(reps=1):
    key = ("nc", reps)
    if key in _prog_cache:
        return _prog_cache[key]
    _apply_patch()
    import concourse.bass as bass
    import concourse.mybir as mybir
    import concourse.tile as tile

    f32 = mybir.dt.float32
    f32r = mybir.dt.float32r
    Relu = mybir.ActivationFunctionType.Relu
    Exp = mybir.ActivationFunctionType.Exp
    AX = mybir.AxisListType.X

    nc = bass.Bass()

    x_ext = nc.dram_tensor("x_sh", [KC, P, RIN, WP], f32r, kind="ExternalInput")
    w_ext = nc.dram_tensor("w_sh", [KC, NTAP, 2, P, P], f32r, kind="ExternalInput")
    wq_ext = nc.dram_tensor("wq_t", [2, P, AQ], f32r, kind="ExternalInput")
    wk_ext = nc.dram_tensor("wk_t", [2, P, AQ], f32r, kind="ExternalInput")
    wv_ext = nc.dram_tensor("wv_t", [2, P, CO], f32r, kind="ExternalInput")
    bnb_ext = nc.dram_tensor("bnb", [2, P, 1], f32, kind="ExternalInput")
    id_ext = nc.dram_tensor("ident", [P, P], f32r, kind="ExternalInput")

    r_out = nc.dram_tensor("r_out", [2, P, HH, W], f32, kind="ExternalOutput")
    q_out = nc.dram_tensor("q_out", [AQ, HH, W], f32, kind="ExternalOutput")
    k_out = nc.dram_tensor("k_out", [AQ, HH, W], f32, kind="ExternalOutput")
    e_out = nc.dram_tensor("e_out", [10, P, NA], f32, kind="ExternalOutput")
    vs_out = nc.dram_tensor("vs_out", [2, P, 10, W], f32, kind="ExternalOutput")
    fin_out = nc.dram_tensor("fin_out", [2, P, 10, W], f32, kind="ExternalOutput")

    with tile.TileContext(nc) as tc:
      for _rep in range(reps):
        with tc.tile_pool(name="persist", bufs=1) as pers:
            wq_sb = [
                pers.tile([P, AQ], f32r, tag=f"wq{c}", name=f"wq_sb{c}")
                for c in range(2)
            ]
            wk_sb = [
                pers.tile([P, AQ], f32r, tag=f"wk{c}", name=f"wk_sb{c}")
                for c in range(2)
            ]
            wv_sb = [
                pers.tile([P, CO], f32r, tag=f"wv{c}", name=f"wv_sb{c}")
                for c in range(2)
            ]
            bnb_sb = [
                pers.tile([P, 1], f32, tag=f"bnb{c}", name=f"bnb_sb{c}")
                for c in range(2)
            ]
            id_sb = pers.tile([P, P], f32r, tag="ident")
            qa = pers.tile([AQ, NA], f32r, tag="qa")
            ka = pers.tile([AQ, NA], f32r, tag="ka")
            vT = pers.tile([P, 10, CO], f32r, tag="vT")
            rattn = pers.tile([P, 2, 10, W], f32r, tag="rattn")

            for c in range(2):
                nc.sync.dma_start(wq_sb[c][:], wq_ext[c])
                nc.sync.dma_start(wk_sb[c][:], wk_ext[c])
                nc.sync.dma_start(wv_sb[c][:], wv_ext[c])
                nc.sync.dma_start(bnb_sb[c][:], bnb_ext[c])
            nc.sync.dma_start(id_sb[:], id_ext[:])

            # ---------------- conv + projections ----------------
            with (
                tc.tile_pool(name="wpool", bufs=1) as wpool,
                tc.tile_pool(name="xpool", bufs=2) as xpool,
                tc.tile_pool(name="accpool", bufs=1) as accpool,
                tc.tile_pool(name="rrpool", bufs=2) as rrpool,
                tc.tile_pool(name="stage", bufs=2) as stage,
                tc.tile_pool(name="cps", bufs=3, space="PSUM") as cps,
                tc.tile_pool(name="qkps", bufs=2, space="PSUM") as qkps,
                tc.tile_pool(name="vps", bufs=1, space="PSUM") as vps,
            ):
                racc = accpool.tile([P, 2, HH, W], f32, tag="racc")

                for p in range(2):
                    w_sb = wpool.tile([P, KCH, NTAP, 2, P], f32r, tag="w")
                    nc.sync.dma_start(
                        w_sb[:],
                        w_ext[p * KCH : (p + 1) * KCH].rearrange(
                            "k t c i o -> i k t c o"
                        ),
                    )
                    for blk in range(HH // 4):
                        s = blk * 4
                        x_sb = xpool.tile([P, KCH, 6, WP], f32r, tag="x")
                        nc.sync.dma_start(
                            x_sb[:],
                            x_ext[p * KCH : (p + 1) * KCH, :, s : s + 6, :].rearrange(
                                "k p r c -> p k r c"
                            ),
                        )
                        rr = {}
                        for cc in range(2):
                            ps = cps.tile([P, 4, W], f32, tag="cps")
                            for k in range(KCH):
                                for tap in range(NTAP):
                                    dy, dx = divmod(tap, 3)
                                    nc.tensor.matmul(
                                        ps[:],
                                        w_sb[:, k, tap, cc, :],
                                        x_sb[:, k, dy : dy + 4, dx : dx + W],
                                        start=(k == 0 and tap == 0),
                                        stop=(k == KCH - 1 and tap == NTAP - 1),
                                    )
                            if p == 0:
                                nc.vector.tensor_copy(racc[:, cc, s : s + 4, :], ps[:])
                            else:
                                nc.vector.tensor_add(
                                    ps[:], ps[:], racc[:, cc, s : s + 4, :]
                                )
                                rrt = rrpool.tile([P, 4, W], f32r, tag=f"rr{cc}")
                                nc.scalar.activation(
                                    rrt[:], ps[:], Relu, bias=bnb_sb[cc][:]
                                )
                                nc.sync.dma_start(
                                    r_out[cc, :, s : s + 4, :],
                                    rrt[:].bitcast(f32),
                                )
                                rr[cc] = rrt

                        if p == 1:
                            # q/k projections for these 4 rows
                            for (proj_w, proj_out, attn_sb, tagn) in (
                                (wq_sb, q_out, qa, "q"),
                                (wk_sb, k_out, ka, "k"),
                            ):
                                prps = qkps.tile([AQ, 4, W], f32, tag="qkps")
                                for cc in range(2):
                                    nc.tensor.matmul(
                                        prps[:],
                                        proj_w[cc][:],
                                        rr[cc][:],
                                        start=(cc == 0),
                                        stop=(cc == 1),
                                    )
                                pst = stage.tile([AQ, 4, W], f32r, tag=f"{tagn}st")
                                nc.vector.tensor_copy(pst[:], prps[:])
                                nc.sync.dma_start(
                                    proj_out[:, s : s + 4, :], pst[:].bitcast(f32)
                                )
                                # attention slice rows [10,20)
                                ov0 = max(s, 10)
                                ov1 = min(s + 4, 20)
                                if ov0 < ov1:
                                    nov = ov1 - ov0
                                    nc.vector.tensor_copy(
                                        attn_sb[
                                            :, (ov0 - 10) * W : (ov1 - 10) * W
                                        ],
                                        pst[:, ov0 - s : ov1 - s, :]
                                        .bitcast(f32)
                                        .rearrange("p r w -> p (r w)"),
                                    )
                            ov0 = max(s, 10)
                            ov1 = min(s + 4, 20)
                            if ov0 < ov1:
                                nov = ov1 - ov0
                                i0 = ov0 - s
                                a0 = ov0 - 10
                                # r slice for the residual add
                                for cc in range(2):
                                    nc.vector.tensor_copy(
                                        rattn[:, cc, a0 : a0 + nov, :],
                                        rr[cc][:, i0 : i0 + nov, :].bitcast(f32),
                                    )
                                # vs (natural layout) for these rows
                                for oc in range(2):
                                    vsps = vps.tile([P, 4, W], f32, tag="vsps")
                                    for cc in range(2):
                                        nc.tensor.matmul(
                                            vsps[:, :nov, :],
                                            wv_sb[cc][:, oc * P : (oc + 1) * P],
                                            rr[cc][:, i0 : i0 + nov, :],
                                            start=(cc == 0),
                                            stop=(cc == 1),
                                        )
                                    vst = stage.tile([P, 4, W], f32, tag="vst")
                                    nc.vector.tensor_copy(
                                        vst[:, :nov, :], vsps[:, :nov, :]
                                    )
                                    nc.sync.dma_start(
                                        vs_out[oc, :, a0 : a0 + nov, :],
                                        vst[:, :nov, :],
                                    )
                                # v^T rows for the output matmul
                                for j in range(nov):
                                    vtps = vps.tile([P, CO], f32, tag="vtps")
                                    for cc in range(2):
                                        nc.tensor.matmul(
                                            vtps[:],
                                            rr[cc][:, i0 + j, :],
                                            wv_sb[cc][:],
                                            start=(cc == 0),
                                            stop=(cc == 1),
                                        )
                                    nc.vector.tensor_copy(vT[:, a0 + j, :], vtps[:])

            # ---------------- attention ----------------
            MBLKS = [(0, 512), (512, 512), (1024, 256)]
            with (
                tc.tile_pool(name="apool", bufs=1) as apool,
                tc.tile_pool(name="stats", bufs=4) as stats,
                tc.tile_pool(name="fstage", bufs=2) as fstage,
                tc.tile_pool(name="aps", bufs=2, space="PSUM") as aps,
                tc.tile_pool(name="trps_pool", bufs=2, space="PSUM") as trps_pool,
                tc.tile_pool(name="ops_pool", bufs=2, space="PSUM") as ops_pool,
            ):
                e_sb = apool.tile([P, 10, NA], f32, tag="e_sb")
                a_sb = apool.tile([P, 10, NA], f32r, tag="a_sb")
                aT = apool.tile([P, 10, NA], f32r, tag="aT")

                for nchk in range(10):
                    for (m0, mw) in MBLKS:
                        eps_t = aps.tile([P, 512], f32, tag="eps")
                        nc.tensor.matmul(
                            eps_t[:, :mw],
                            qa[:, nchk * P : (nchk + 1) * P],
                            ka[:, m0 : m0 + mw],
                            start=True,
                            stop=True,
                        )
                        nc.vector.tensor_copy(
                            e_sb[:, nchk, m0 : m0 + mw], eps_t[:, :mw]
                        )
                    nc.sync.dma_start(e_out[nchk], e_sb[:, nchk, :])
                    negmax = stats.tile([P, 1], f32, tag="negmax")
                    nc.vector.reduce_max(
                        negmax[:], e_sb[:, nchk, :], axis=AX, negate=True
                    )
                    nc.scalar.activation(
                        a_sb[:, nchk, :], e_sb[:, nchk, :], Exp, bias=negmax[:]
                    )
                    ssum = stats.tile([P, 1], f32, tag="ssum")
                    nc.vector.reduce_sum(ssum[:], a_sb[:, nchk, :].bitcast(f32), axis=AX)
                    sinv = stats.tile([P, 1], f32, tag="sinv")
                    nc.vector.reciprocal(sinv[:], ssum[:])
                    nc.vector.tensor_scalar_mul(
                        a_sb[:, nchk, :], a_sb[:, nchk, :].bitcast(f32), sinv[:]
                    )

                for nchk in range(10):
                    for mchk in range(10):
                        trps = trps_pool.tile([P, P], f32r, tag="trps")
                        nc.tensor.transpose(
                            trps[:],
                            a_sb[:, nchk, mchk * P : (mchk + 1) * P],
                            id_sb[:],
                        )
                        nc.vector.tensor_copy(
                            aT[:, mchk, nchk * P : (nchk + 1) * P],
                            trps[:].bitcast(f32),
                        )

                NBLKS = [(0, 512, 0, 4), (512, 512, 4, 4), (1024, 256, 8, 2)]
                for cc in range(2):
                    for (n0, nw, row0, nrows) in NBLKS:
                        ops_t = ops_pool.tile([P, 512], f32, tag="ops")
                        for mchk in range(10):
                            nc.tensor.matmul(
                                ops_t[:, :nw],
                                vT[:, mchk, cc * P : (cc + 1) * P],
                                aT[:, mchk, n0 : n0 + nw],
                                start=(mchk == 0),
                                stop=(mchk == 9),
                            )
                        fo = fstage.tile([P, 512], f32, tag="fo")
                        nc.vector.tensor_add(
                            fo[:, :nw],
                            ops_t[:, :nw],
                            rattn[:, cc, row0 : row0 + nrows, :]
                            .bitcast(f32)
                            .rearrange("p r w -> p (r w)"),
                        )
                        nc.sync.dma_start(
                            fin_out[cc, :, row0 : row0 + nrows, :],
                            fo[:, :nw].rearrange("p (r w) -> p r w", w=W),
                        )

    _prog_cache[key] = nc
    return nc


# ---------------- host side ----------------------------------------------

TRACE = False  # set by test harness for HW timing; harness default is off
REPS = 1  # test harness sets >1 to measure device time by differencing


def kernel(**inputs):
    x = np.asarray(inputs["x"], dtype=np.float32)
    w_collect = np.asarray(inputs["w_collect"], dtype=np.float32)
    bn_gamma = np.asarray(inputs["bn_gamma"], dtype=np.float32)
    bn_beta = np.asarray(inputs["bn_beta"], dtype=np.float32)
    bn_mean = np.asarray(inputs["bn_mean"], dtype=np.float32)
    bn_var = np.asarray(inputs["bn_var"], dtype=np.float32)
    w_q = np.asarray(inputs["w_q"], dtype=np.float32)
    w_k = np.asarray(inputs["w_k"], dtype=np.float32)
    w_v = np.asarray(inputs["w_v"], dtype=np.float32)

    nc = _build_program(REPS)
    from concourse.bass_utils import run_bass_kernel_spmd

    # ---- weight prep ----
    inv_std = 1.0 / np.sqrt(bn_var.astype(np.float64) + EPS)
    scale = (bn_gamma.astype(np.float64) * inv_std).astype(np.float32)
    shift = (
        bn_beta.astype(np.float64)
        - bn_mean.astype(np.float64) * bn_gamma.astype(np.float64) * inv_std
    ).astype(np.float32)
    w_eff = w_collect * scale[:, None, None, None]
    # [co, ci, dy, dx] -> [kc, tap, cc, ci_in, co_in]
    w_lhsT = _round_f32r(
        np.ascontiguousarray(
            w_eff.reshape(2, P, KC, P, 3, 3).transpose(2, 4, 5, 0, 3, 1)
        ).reshape(KC, NTAP, 2, P, P)
    )
    wq_t = _round_f32r(np.ascontiguousarray(w_q.T).reshape(2, P, AQ))
    wk_t = _round_f32r(np.ascontiguousarray(w_k.T).reshape(2, P, AQ))
    wv_t = _round_f32r(np.ascontiguousarray(w_v.T).reshape(2, P, CO))
    bnb = np.ascontiguousarray(shift.reshape(2, P, 1))
    ident = np.eye(P, dtype=np.float32)

    # ---- x shards ----
    xr = x.reshape(B, KC, P, H, W)
    xp = np.zeros((B, 2, KC, P, RIN, WP), np.float32)
    xp[:, 0, :, :, 1:RIN, 1 : W + 1] = xr[:, :, :, 0 : HH + 1, :]
    xp[:, 1, :, :, 0 : RIN - 1, 1 : W + 1] = xr[:, :, :, HH - 1 : H, :]
    xp = _round_f32r(xp)

    in_maps = []
    for c in range(NCORES):
        b, half = c // 2, c % 2
        in_maps.append(
            {
                "x_sh": np.ascontiguousarray(xp[b, half]),
                "w_sh": w_lhsT,
                "wq_t": wq_t,
                "wk_t": wk_t,
                "wv_t": wv_t,
                "bnb": bnb,
                "ident": ident,
            }
        )

    _run = run_bass_kernel_spmd(
        nc,
        in_maps,
        core_ids=list(range(NCORES)),
        trace=TRACE,
        trace_cores=list(range(NCORES)) if TRACE else None,
        stitch_traces=TRACE,
    )
    _prog_cache["last_run"] = _run
    res = _run.results

    # ---- assemble full outputs ----
    r_full = np.empty((B, CO, H, W), np.float32)
    pq = np.empty((B, AQ, H, W), np.float32)
    pk = np.empty((B, AQ, H, W), np.float32)
    energy = np.empty((B, NA, NA), np.float32)
    vs = np.empty((B, CO, NA), np.float32)
    fin = np.empty((B, CO, 10, W), np.float32)
    for c in range(NCORES):
        b, half = c // 2, c % 2
        sl = slice(half * HH, (half + 1) * HH)
        r_full[b, :, sl] = res[c]["r_out"].reshape(CO, HH, W)
        pq[b, :, sl] = res[c]["q_out"]
        pk[b, :, sl] = res[c]["k_out"]
        if half == 0:
            energy[b] = res[c]["e_out"].reshape(NA, NA)
            vs[b] = res[c]["vs_out"].reshape(CO, NA)
            fin[b] = res[c]["fin_out"].reshape(CO, 10, W)

    final = r_full.copy()
    final[:, :, 10:20, :] = fin
    return (final, energy, pq, pk, vs, r_full)
